# revision 20
# baseline (speedup 1.0000x reference)
"""Multi-head attention (GAttention) on 8 trn2 NeuronCores.

Reference computation (per batch b):
    q = x @ w_qkv.T            -> [N, 768], heads of 64
    attn = softmax(q k^T / 8)  -> per head [N, M]
    out_h = attn @ v           -> [N, 64]
    out = concat(out_h) @ w_proj.T + b_proj

Sharding: 24 (b, head) units over 8 cores -> each core gets one batch b and
3 heads. Each core computes its heads' attention plus its partial
projection sum [N, 768]; host adds the 4 partials per batch + bias.

Per-core device pipeline:
  1. qproj (f32r): qT_dup[128, N] per head = [wq_h | wq_h]^T x^T; the
     duplicated column block makes rows 64:128 a copy of rows 0:64, which
     feeds the row-packed S^T matmuls.
  2. attention (bf16 operands, f32 PSUM), 6 (head, n-half) units; per key
     m-tile PAIR (2 x 128 keys, PE row groups 0/64 run concurrently):
       S^T = k q^T   -> PSUM [128, 2, 512] per n-chunk (tile A/B)
       expT = exp(0.125 S^T) -> SBUF bf16 (ACT, fused scale)
       AV: av[128, 1024] += v_aug^T expT   (accumulate over all 16 m-tiles)
     v_aug = [v_h | ones*64] so av rows 64:128 hold the softmax denominator.
  3. normalize: outTn (both partition halves) = av[0:64] * recip(av[64:128])
  4. proj (f32r): row-packed n-tile pairs, PSUM accumulates the 3 heads.
"""
import numpy as np
import ml_dtypes
from contextlib import ExitStack

import concourse.bass as bass
import concourse.mybir as mybir
import concourse.tile as tile
from concourse import bacc
from concourse.bass_utils import run_bass_kernel_spmd

B, N, DIM = 2, 2048, 768
H, D = 12, 64
M = 2048
NCORES = 8
HPC = 3            # heads per core
NT = N // 128      # 16 query tiles
MT = M // 128      # 16 key tiles
MP = MT // 2       # 8 key-tile pairs
CT = DIM // 128    # 6 contraction tiles for qproj
NHALF = 1024       # AV psum n-granularity
F32 = mybir.dt.float32
F32R = mybir.dt.float32r
BF16 = mybir.dt.bfloat16

_cached = {}

# dtype config: "fast" = bf16 attention+qproj, "mid" = f32r qproj + bf16 attn,
# "safe" = all f32r
import os
QUALITY = os.environ.get("KQ", "fast")
QP_DT = BF16 if QUALITY == "fast" else F32R
AT_DT = F32R if QUALITY == "safe" else BF16


def build_program():
    nc = bacc.Bacc("TRN2", target_bir_lowering=False, debug=False)
    xT_d = nc.dram_tensor("xT", [DIM, N], QP_DT, kind="ExternalInput")
    wq_d = nc.dram_tensor("wq", [HPC, DIM, 128], QP_DT, kind="ExternalInput")
    kT_d = nc.dram_tensor("kT", [128, HPC, MP, 128], AT_DT,
                          kind="ExternalInput")
    va_d = nc.dram_tensor("va", [HPC, M, 128], AT_DT, kind="ExternalInput")
    wp_d = nc.dram_tensor("wp", [128, HPC, DIM], F32R, kind="ExternalInput")
    out_d = nc.dram_tensor("out", [N, DIM], F32, kind="ExternalOutput")

    with tile.TileContext(nc) as tc, ExitStack() as ctx:
        big = ctx.enter_context(tc.tile_pool(name="big", bufs=1))
        expp = ctx.enter_context(tc.tile_pool(name="expp", bufs=4))
        stg = ctx.enter_context(tc.tile_pool(name="stg", bufs=3))

        # persistent SBUF tensors; DMA order = consumption order:
        # wq (small, needed by first matmul), xT tiles (qproj streams them),
        # then per-head k/v in attention-unit order, wp last
        wq_t = big.tile([128, HPC, CT, 128], QP_DT)
        nc.sync.dma_start(
            wq_t[:], wq_d.rearrange("h (c p) d -> p h c d", p=128))
        xT_t = [big.tile([128, N], QP_DT, name=f"xT{c}", tag=f"xT{c}")
                for c in range(CT)]
        for c in range(CT):
            nc.sync.dma_start(xT_t[c][:], xT_d[c * 128:(c + 1) * 128, :])
        kT_t = big.tile([128, HPC, MP, 128], AT_DT)
        va_t = big.tile([128, HPC, MT, 128], AT_DT)
        for h in range(HPC):
            nc.sync.dma_start(kT_t[:, h, :, :], kT_d[:, h, :, :])
            nc.sync.dma_start(
                va_t[:, h, :, :],
                va_d[h].rearrange("(t p) e -> p t e", p=128))
        wp_t = big.tile([128, HPC, DIM], F32R)
        nc.sync.dma_start(wp_t[:], wp_d[:])
        qT_t = big.tile([128, HPC, N], AT_DT)
        outTn_t = big.tile([128, HPC, N], F32R)

        # phase 1: q projection; wq has the head slice duplicated so rows
        # 64:128 of qT_t replicate rows 0:64
        with tc.tile_pool(name="qp_ps", bufs=1, space="PSUM") as qp_ps:
            for h in range(HPC):
                qp = qp_ps.tile([128, N], F32)
                for c in range(CT):
                    for ch in range(N // 512):
                        nc.tensor.matmul(
                            qp[:, ch * 512:(ch + 1) * 512],
                            wq_t[:, h, c, :],
                            xT_t[c][:, ch * 512:(ch + 1) * 512],
                            start=(c == 0), stop=(c == CT - 1),
                        )
                nc.vector.tensor_copy(qT_t[:, h, :], qp[:])

        # phase 2+3: attention in 6 (head, n-half) units, half-major order,
        # with the projection for a completed n-half interleaved into the
        # remaining attention iterations. m-tile pairs are row-packed on the
        # PE (row groups 0 and 64); AV matmuls lag 2 iterations so the
        # in-order PE queue never stalls on the EXP wait.
        with tc.tile_pool(name="st_ps", bufs=2, space="PSUM") as st_ps, \
             tc.tile_pool(name="av_ps", bufs=1, space="PSUM") as av_ps, \
             tc.tile_pool(name="pj_ps", bufs=1, space="PSUM") as pj_ps:
            av_by_unit = {}
            normed = set()
            proj_ready = []

            def _av(pend):
                unit, et, p, cc, first, last = pend[:6]
                av = av_by_unit[unit]
                nc.tensor.matmul(
                    av[:, cc * 512:(cc + 1) * 512],
                    va_t[:, unit[0], 2 * p, :], et[:, 0, :],
                    start=first, stop=False,
                )
                nc.tensor.matmul(
                    av[:, cc * 512:(cc + 1) * 512],
                    va_t[:, unit[0], 2 * p + 1, :], et[:, 1, :],
                    start=False, stop=last,
                )

            def _norm(unit):
                # copy numerator+denominator out fast to release the av slot;
                # reciprocal + normalize run off the critical path
                h, half = unit
                av = av_by_unit[unit]
                dn = expp.tile([64, NHALF], F32, tag="dn", name="dn")
                nc.vector.tensor_copy(dn[:], av[64:128, :])
                nm = expp.tile([64, NHALF], F32, tag="nm", name="nm")
                nc.vector.tensor_copy(nm[:], av[0:64, :])
                rs = expp.tile([64, NHALF], F32, tag="rs", name="rs")
                nc.vector.reciprocal_approx_fast(rs[:], dn[:])
                nsl = slice(half * NHALF, (half + 1) * NHALF)
                nc.vector.tensor_mul(outTn_t[0:64, h, nsl], nm[:], rs[:])
                nc.vector.tensor_mul(outTn_t[64:128, h, nsl], nm[:], rs[:])
                normed.add(unit)
                if all((hh, half) in normed for hh in range(HPC)):
                    ntiles_per_half = NHALF // 128
                    proj_ready.extend(
                        range(half * ntiles_per_half,
                              (half + 1) * ntiles_per_half))

            def _proj_tile(ni):
                pp = pj_ps.tile([128, 2, 512], F32, tag="pp", name="pp")
                for hh in range(HPC):
                    for oc in range(2):
                        nc.tensor.matmul(
                            pp[:, oc, 0:384],
                            outTn_t[0:64, hh, ni * 128:(ni + 1) * 128],
                            wp_t[0:64, hh, oc * 384:(oc + 1) * 384],
                            start=(hh == 0), stop=(hh == HPC - 1),
                        )
                os_t = stg.tile([128, DIM], F32, tag="os", name="os")
                nc.vector.tensor_copy(os_t[:, 0:384], pp[:, 0, 0:384])
                nc.vector.tensor_copy(os_t[:, 384:768], pp[:, 1, 0:384])
                nc.sync.dma_start(out_d[ni * 128:(ni + 1) * 128, :], os_t[:])

            pend = []
            LAG = 2

            def _flush(limit):
                while len(pend) > limit:
                    pd = pend.pop(0)
                    _av(pd)
                    if pd[6]:
                        _norm(pd[0])

            iters = [(h, half, p, cc)
                     for half in range(N // NHALF) for h in range(HPC)
                     for p in range(MP) for cc in range(NHALF // 512)]
            for idx, (h, half, p, cc) in enumerate(iters):
                unit = (h, half)
                if unit not in av_by_unit:
                    av_by_unit[unit] = av_ps.tile(
                        [128, NHALF], F32, tag="av", name="av")
                n0 = half * NHALF + cc * 512
                st = st_ps.tile([128, 2, 512], F32, tag="st", name="st")
                nc.tensor.matmul(
                    st[:, 0, :], kT_t[0:64, h, p, :],
                    qT_t[0:64, h, n0:n0 + 512],
                    start=True, stop=True, tile_position=(0, 0),
                )
                nc.tensor.matmul(
                    st[:, 1, :], kT_t[64:128, h, p, :],
                    qT_t[64:128, h, n0:n0 + 512],
                    start=True, stop=True, tile_position=(64, 0),
                )
                _flush(LAG - 1)
                if proj_ready and idx % 6 == 0:
                    _proj_tile(proj_ready.pop(0))
                et = expp.tile([128, 2, 512], AT_DT, tag="et", name="et")
                nc.scalar.activation(
                    et[:], st[:], mybir.ActivationFunctionType.Exp,
                    scale=float(D) ** -0.5,
                )
                pend.append((unit, et, p, cc, p == 0, p == MP - 1,
                             p == MP - 1 and cc == NHALF // 512 - 1))
            _flush(0)
            while proj_ready:
                _proj_tile(proj_ready.pop(0))

    nc.compile()
    return nc


def build_in_maps(x, k, v, w_qkv, w_proj):
    x = np.asarray(x, dtype=np.float32)
    k = np.asarray(k, dtype=np.float32)
    v = np.asarray(v, dtype=np.float32)
    wqT = np.ascontiguousarray(np.asarray(w_qkv, np.float32).T)   # [C, 768]
    wpT = np.ascontiguousarray(np.asarray(w_proj, np.float32).T)  # [768, 768]

    in_maps = []
    for core in range(NCORES):
        b = core // 4
        hs = [3 * (core % 4) + i for i in range(HPC)]
        qp_np = ml_dtypes.bfloat16 if QUALITY == "fast" else np.float32
        at_np = np.float32 if QUALITY == "safe" else ml_dtypes.bfloat16
        xT = np.ascontiguousarray(x[b].T.astype(qp_np))
        # duplicated head slice -> qT rows 64:128 == rows 0:64
        wq = np.stack([
            np.concatenate([wqT[:, 64 * h:64 * (h + 1)]] * 2, axis=1)
            for h in hs]).astype(qp_np)                          # [3, DIM, 128]
        # kT layout [128, HPC, MP, 128]: rows 0:64 = head-dim of even m-tile,
        # rows 64:128 = head-dim of odd m-tile of each pair
        kb = k[b, hs].astype(at_np)                              # [3, M, D]
        kT = np.empty((128, HPC, MP, 128), dtype=at_np)
        for hi in range(HPC):
            for p in range(MP):
                kT[0:64, hi, p, :] = kb[hi, 256 * p:256 * p + 128, :].T
                kT[64:128, hi, p, :] = kb[hi, 256 * p + 128:256 * p + 256, :].T
        va = np.ones((HPC, M, 128), dtype=at_np)
        va[:, :, :D] = v[b, hs].astype(at_np)       # [3, M, 128]
        # wp duplicated on both partition halves for row-packed proj
        wp = np.empty((128, HPC, DIM), dtype=np.float32)
        for hi, h in enumerate(hs):
            wp[0:64, hi, :] = wpT[64 * h:64 * (h + 1), :]
            wp[64:128, hi, :] = wpT[64 * h:64 * (h + 1), :]
        in_maps.append({"xT": xT, "wq": wq,
                        "kT": np.ascontiguousarray(kT),
                        "va": np.ascontiguousarray(va),
                        "wp": np.ascontiguousarray(wp)})
    return in_maps


def kernel(x, k, v, w_qkv, w_proj, b_proj):
    b_proj = np.asarray(b_proj, dtype=np.float32)

    if "nc" not in _cached:
        _cached["nc"] = build_program()
    nc = _cached["nc"]

    in_maps = build_in_maps(x, k, v, w_qkv, w_proj)
    res = run_bass_kernel_spmd(nc, in_maps, core_ids=list(range(NCORES)))

    out = np.empty((B, N, DIM), dtype=np.float32)
    for b in range(B):
        acc = np.zeros((N, DIM), dtype=np.float64)
        for core in range(4 * b, 4 * b + 4):
            acc += res.results[core]["out"]
        out[b] = (acc + b_proj).astype(np.float32)
    return out


# revision 21
# speedup vs baseline: 1.0683x; 1.0683x over previous
"""Multi-head attention (GAttention) on 8 trn2 NeuronCores.

Reference computation (per batch b):
    q = x @ w_qkv.T            -> [N, 768], heads of 64
    attn = softmax(q k^T / 8)  -> per head [N, M]
    out_h = attn @ v           -> [N, 64]
    out = concat(out_h) @ w_proj.T + b_proj

Sharding: 24 (b, head) units over 8 cores -> each core gets one batch b and
3 heads. Each core computes its heads' attention plus its partial
projection sum [N, 768]; host adds the 4 partials per batch + bias.

Per-core device pipeline:
  1. qproj (f32r): qT_dup[128, N] per head = [wq_h | wq_h]^T x^T; the
     duplicated column block makes rows 64:128 a copy of rows 0:64, which
     feeds the row-packed S^T matmuls.
  2. attention (bf16 operands, f32 PSUM), 6 (head, n-half) units; per key
     m-tile PAIR (2 x 128 keys, PE row groups 0/64 run concurrently):
       S^T = k q^T   -> PSUM [128, 2, 512] per n-chunk (tile A/B)
       expT = exp(0.125 S^T) -> SBUF bf16 (ACT, fused scale)
       AV: av[128, 1024] += v_aug^T expT   (accumulate over all 16 m-tiles)
     v_aug = [v_h | ones*64] so av rows 64:128 hold the softmax denominator.
  3. normalize: outTn (both partition halves) = av[0:64] * recip(av[64:128])
  4. proj (f32r): row-packed n-tile pairs, PSUM accumulates the 3 heads.
"""
import numpy as np
import ml_dtypes
from contextlib import ExitStack

import concourse.bass as bass
import concourse.mybir as mybir
import concourse.tile as tile
from concourse import bacc
from concourse.bass_utils import run_bass_kernel_spmd

B, N, DIM = 2, 2048, 768
H, D = 12, 64
M = 2048
NCORES = 8
HPC = 3            # heads per core
NT = N // 128      # 16 query tiles
MT = M // 128      # 16 key tiles
MP = MT // 2       # 8 key-tile pairs
CT = DIM // 128    # 6 contraction tiles for qproj
NHALF = 1024       # AV psum n-granularity
F32 = mybir.dt.float32
F32R = mybir.dt.float32r
BF16 = mybir.dt.bfloat16

_cached = {}

# dtype config: "fast" = bf16 attention+qproj, "mid" = f32r qproj + bf16 attn,
# "safe" = all f32r
import os
QUALITY = os.environ.get("KQ", "fast")
QP_DT = BF16 if QUALITY == "fast" else F32R
AT_DT = F32R if QUALITY == "safe" else BF16


def build_program():
    nc = bacc.Bacc("TRN2", target_bir_lowering=False, debug=False)
    xT_d = nc.dram_tensor("xT", [DIM, N], QP_DT, kind="ExternalInput")
    wq_d = nc.dram_tensor("wq", [HPC, DIM, 128], QP_DT, kind="ExternalInput")
    kT_d = nc.dram_tensor("kT", [128, HPC, MP, 128], AT_DT,
                          kind="ExternalInput")
    va_d = nc.dram_tensor("va", [HPC, M, 128], AT_DT, kind="ExternalInput")
    wp_d = nc.dram_tensor("wp", [128, HPC, DIM], F32R, kind="ExternalInput")
    out_d = nc.dram_tensor("out", [N, DIM], F32, kind="ExternalOutput")

    with tile.TileContext(nc) as tc, ExitStack() as ctx:
        big = ctx.enter_context(tc.tile_pool(name="big", bufs=1))
        expp = ctx.enter_context(tc.tile_pool(name="expp", bufs=4))
        stg = ctx.enter_context(tc.tile_pool(name="stg", bufs=3))

        # persistent SBUF tensors; DMA order = consumption order: wq and
        # head-0 k/v first (cheap, unblock the first attention unit), then
        # the xT stream that paces qproj, then the rest
        wq_t = big.tile([128, HPC, CT, 128], QP_DT)
        nc.sync.dma_start(
            wq_t[:], wq_d.rearrange("h (c p) d -> p h c d", p=128))
        kT_t = big.tile([128, HPC, MP, 128], AT_DT)
        va_t = big.tile([128, HPC, MT, 128], AT_DT)
        nc.sync.dma_start(kT_t[:, 0, :, :], kT_d[:, 0, :, :])
        nc.sync.dma_start(va_t[:, 0, :, :],
                          va_d[0].rearrange("(t p) e -> p t e", p=128))
        xT_t = [big.tile([128, N], QP_DT, name=f"xT{c}", tag=f"xT{c}")
                for c in range(CT)]
        for c in range(CT):
            nc.sync.dma_start(xT_t[c][:], xT_d[c * 128:(c + 1) * 128, :])
        for h in range(1, HPC):
            nc.sync.dma_start(kT_t[:, h, :, :], kT_d[:, h, :, :])
            nc.sync.dma_start(va_t[:, h, :, :],
                              va_d[h].rearrange("(t p) e -> p t e", p=128))
        wp_t = big.tile([128, HPC, DIM], F32R)
        nc.sync.dma_start(wp_t[:], wp_d[:])
        qT_t = big.tile([128, HPC, N], AT_DT)
        outTn_t = big.tile([128, HPC, N], F32R)

        # phase 1: q projection; wq has the head slice duplicated so rows
        # 64:128 of qT_t replicate rows 0:64
        with tc.tile_pool(name="qp_ps", bufs=1, space="PSUM") as qp_ps:
            for h in range(HPC):
                qp = qp_ps.tile([128, N], F32)
                for c in range(CT):
                    for ch in range(N // 512):
                        nc.tensor.matmul(
                            qp[:, ch * 512:(ch + 1) * 512],
                            wq_t[:, h, c, :],
                            xT_t[c][:, ch * 512:(ch + 1) * 512],
                            start=(c == 0), stop=(c == CT - 1),
                        )
                nc.vector.tensor_copy(qT_t[:, h, :], qp[:])

        # phase 2: attention in 6 (head, n-half) units; m-tile pairs are
        # row-packed on the PE (row groups 0 and 64). The AV matmuls for
        # iteration i are issued AFTER iteration i+1's S^T so the in-order
        # PE queue never stalls behind the EXP wait.
        with tc.tile_pool(name="st_ps", bufs=3, space="PSUM") as st_ps, \
             tc.tile_pool(name="av_ps", bufs=1, space="PSUM") as av_ps:
            av_by_unit = {}

            def _av(pend):
                unit, et, p, cc, first, last = pend[:6]
                av = av_by_unit[unit]
                nc.tensor.matmul(
                    av[:, cc * 512:(cc + 1) * 512],
                    va_t[:, unit[0], 2 * p, :], et[:, 0, :],
                    start=first, stop=False,
                )
                nc.tensor.matmul(
                    av[:, cc * 512:(cc + 1) * 512],
                    va_t[:, unit[0], 2 * p + 1, :], et[:, 1, :],
                    start=False, stop=last,
                )

            def _norm(unit):
                # copy numerator+denominator out fast to release the av slot;
                # reciprocal + normalize then run off the critical path
                h, half = unit
                av = av_by_unit[unit]
                dn = expp.tile([64, NHALF], F32, tag="dn", name="dn")
                nc.vector.tensor_copy(dn[:], av[64:128, :])
                nm = expp.tile([64, NHALF], F32, tag="nm", name="nm")
                nc.vector.tensor_copy(nm[:], av[0:64, :])
                rs = expp.tile([64, NHALF], F32, tag="rs", name="rs")
                nc.vector.reciprocal_approx_fast(rs[:], dn[:])
                nsl = slice(half * NHALF, (half + 1) * NHALF)
                nc.vector.tensor_mul(
                    outTn_t[0:64, h, nsl], nm[:], rs[:])
                nc.vector.tensor_mul(
                    outTn_t[64:128, h, nsl], nm[:], rs[:])

            iters = [(h, half, p, cc)
                     for half in range(N // NHALF) for h in range(HPC)
                     for p in range(MP) for cc in range(NHALF // 512)]
            pend = []
            LAG = 2

            def _flush(limit):
                while len(pend) > limit:
                    pd = pend.pop(0)
                    _av(pd)
                    if pd[6]:
                        _norm(pd[0])

            for h, half, p, cc in iters:
                unit = (h, half)
                if unit not in av_by_unit:
                    av_by_unit[unit] = av_ps.tile(
                        [128, NHALF], F32, tag="av", name="av")
                n0 = half * NHALF + cc * 512
                st = st_ps.tile([128, 2, 512], F32, tag="st", name="st")
                nc.tensor.matmul(
                    st[:, 0, :], kT_t[0:64, h, p, :],
                    qT_t[0:64, h, n0:n0 + 512],
                    start=True, stop=True, tile_position=(0, 0),
                )
                nc.tensor.matmul(
                    st[:, 1, :], kT_t[64:128, h, p, :],
                    qT_t[64:128, h, n0:n0 + 512],
                    start=True, stop=True, tile_position=(64, 0),
                )
                _flush(LAG - 1)
                et = expp.tile([128, 2, 512], AT_DT, tag="et", name="et")
                nc.scalar.activation(
                    et[:], st[:], mybir.ActivationFunctionType.Exp,
                    scale=float(D) ** -0.5,
                )
                pend.append((unit, et, p, cc, p == 0, p == MP - 1,
                             p == MP - 1 and cc == NHALF // 512 - 1))
            _flush(0)

        # phase 3: projection, row-packed n-tile pairs, PSUM accumulates
        # the 3 heads
        with tc.tile_pool(name="pj_ps", bufs=2, space="PSUM") as pj_ps:
            for nj in range(NT // 2):
                ppa = pj_ps.tile([128, 2, 512], F32, tag="ppa")
                ppb = pj_ps.tile([128, 2, 512], F32, tag="ppb")
                na = 2 * nj * 128
                nb = (2 * nj + 1) * 128
                for h in range(HPC):
                    for oc in range(2):
                        osl = slice(oc * 384, (oc + 1) * 384)
                        nc.tensor.matmul(
                            ppa[:, oc, 0:384],
                            outTn_t[0:64, h, na:na + 128],
                            wp_t[0:64, h, osl],
                            start=(h == 0), stop=(h == HPC - 1),
                            tile_position=(0, 0),
                        )
                        nc.tensor.matmul(
                            ppb[:, oc, 0:384],
                            outTn_t[64:128, h, nb:nb + 128],
                            wp_t[64:128, h, osl],
                            start=(h == 0), stop=(h == HPC - 1),
                            tile_position=(64, 0),
                        )
                for which, pp, nn in ((0, ppa, na), (1, ppb, nb)):
                    os_t = stg.tile([128, DIM], F32, tag="os", name="os")
                    if which == 0:
                        nc.vector.tensor_copy(os_t[:, 0:384], pp[:, 0, 0:384])
                        nc.vector.tensor_copy(os_t[:, 384:768],
                                              pp[:, 1, 0:384])
                    else:
                        nc.scalar.copy(os_t[:, 0:384], pp[:, 0, 0:384])
                        nc.scalar.copy(os_t[:, 384:768], pp[:, 1, 0:384])
                    nc.sync.dma_start(out_d[nn:nn + 128, :], os_t[:])

    nc.compile()
    return nc


def build_in_maps(x, k, v, w_qkv, w_proj):
    x = np.asarray(x, dtype=np.float32)
    k = np.asarray(k, dtype=np.float32)
    v = np.asarray(v, dtype=np.float32)
    wqT = np.ascontiguousarray(np.asarray(w_qkv, np.float32).T)   # [C, 768]
    wpT = np.ascontiguousarray(np.asarray(w_proj, np.float32).T)  # [768, 768]

    in_maps = []
    for core in range(NCORES):
        b = core // 4
        hs = [3 * (core % 4) + i for i in range(HPC)]
        qp_np = ml_dtypes.bfloat16 if QUALITY == "fast" else np.float32
        at_np = np.float32 if QUALITY == "safe" else ml_dtypes.bfloat16
        xT = np.ascontiguousarray(x[b].T.astype(qp_np))
        # duplicated head slice -> qT rows 64:128 == rows 0:64
        wq = np.stack([
            np.concatenate([wqT[:, 64 * h:64 * (h + 1)]] * 2, axis=1)
            for h in hs]).astype(qp_np)                          # [3, DIM, 128]
        # kT layout [128, HPC, MP, 128]: rows 0:64 = head-dim of even m-tile,
        # rows 64:128 = head-dim of odd m-tile of each pair
        kb = k[b, hs].astype(at_np)                              # [3, M, D]
        kT = np.empty((128, HPC, MP, 128), dtype=at_np)
        for hi in range(HPC):
            for p in range(MP):
                kT[0:64, hi, p, :] = kb[hi, 256 * p:256 * p + 128, :].T
                kT[64:128, hi, p, :] = kb[hi, 256 * p + 128:256 * p + 256, :].T
        va = np.ones((HPC, M, 128), dtype=at_np)
        va[:, :, :D] = v[b, hs].astype(at_np)       # [3, M, 128]
        # wp duplicated on both partition halves for row-packed proj
        wp = np.empty((128, HPC, DIM), dtype=np.float32)
        for hi, h in enumerate(hs):
            wp[0:64, hi, :] = wpT[64 * h:64 * (h + 1), :]
            wp[64:128, hi, :] = wpT[64 * h:64 * (h + 1), :]
        in_maps.append({"xT": xT, "wq": wq,
                        "kT": np.ascontiguousarray(kT),
                        "va": np.ascontiguousarray(va),
                        "wp": np.ascontiguousarray(wp)})
    return in_maps


def kernel(x, k, v, w_qkv, w_proj, b_proj):
    b_proj = np.asarray(b_proj, dtype=np.float32)

    if "nc" not in _cached:
        _cached["nc"] = build_program()
    nc = _cached["nc"]

    in_maps = build_in_maps(x, k, v, w_qkv, w_proj)
    res = run_bass_kernel_spmd(nc, in_maps, core_ids=list(range(NCORES)))

    out = np.empty((B, N, DIM), dtype=np.float32)
    for b in range(B):
        acc = np.zeros((N, DIM), dtype=np.float64)
        for core in range(4 * b, 4 * b + 4):
            acc += res.results[core]["out"]
        out[b] = (acc + b_proj).astype(np.float32)
    return out
